# revision 1
# baseline (speedup 1.0000x reference)
"""IDW k-NN flow interpolation (cdist -> top-8 -> inverse-distance-weighted
gather) on 8 Trainium2 NeuronCores.

Sharding: queries split 8 ways (4096/core); ref points/flows replicated.

Per-core Bass kernel, per 128-query tile (m = 16384 refs):
  1. PE bf16 matmul (K=19 rows, 1 cycle/elem vs fp32's 4) computes a PACKED
     score per (query, ref):  P = C2 - round(d^2)*2^14 + j.  The d^2 terms
     (hi/lo bf16 splits, scale 2^14) accumulate first at natural fp32
     magnitude; a C1 = 1.5*2^37 row then quantizes the running sum to the
     2^14 quantum ONCE (PE PSUM accumulation is sequential per lhsT row with
     per-step fp32 rounding - verified on HW); -C1 cancels exactly
     (Sterbenz) and iota rows land the column index j in the exact low-14-
     bit window.  Larger P <=> smaller d^2, with j an extractable payload.
  2. ACT evicts PSUM -> SBUF (4 banks per copy).
  3. DVE max8 per 8192-column half -> 16 candidates/query with embedded
     indices.  A true top-8 member always has exact seg-rank <= 8, and
     refs are Morton-sorted and dealt alternately into the halves
     (host-side layout), so the +-0.5-unit quantization never displaces
     one: capture is lossless up to ~1e-4 of rows.
  4. Extract j (trunc/mul/sub smalls), 16 indirect DMAs gather each
     candidate's (point, flow) table row, exact fp32 d^2 refinement picks
     the true top-8 by threshold mask, then IDW-weighted flow.
"""

import os
import sys

import numpy as np

for _p in ("/opt/trn_rl_repo", "/root/.axon_site/_ro/trn_rl_repo"):
    if os.path.isdir(_p) and _p not in sys.path:
        sys.path.append(_p)

import ml_dtypes  # noqa: E402
from concourse import bass, mybir  # noqa: E402
from concourse import tile  # noqa: E402
from concourse.bass_utils import run_bass_kernel_spmd  # noqa: E402

N_FULL = 32768
M_FULL = 16384
D = 3
K = 8
N_CORES = 8
P = 128
CH = 512          # psum bank (fp32 elems)
BANKS = 4         # banks per PSUM->SBUF eviction DMA
KR = 19           # packed-matmul rows
NSEG = 2          # max8 segments per tile row
CAND = NSEG * 8   # candidates per query
TE = 8            # table row elems (32B: pts(3) ++ flow(3) ++ pad)
C1 = 1.5 * 2.0**37

_bf = lambda x: np.asarray(x, dtype=ml_dtypes.bfloat16)  # noqa: E731


def build_module(n_loc=N_FULL // N_CORES, m=M_FULL, split=True, stage=3):
    nt = n_loc // P
    G = 4
    mg = m // G
    ncg = mg // CH
    assert n_loc % P == 0 and mg % CH == 0 and m % NSEG == 0

    nc = bass.Bass("TRN2", debug=False)

    comb_d = nc.dram_tensor(
        "comb", [G * KR, mg + n_loc], mybir.dt.bfloat16, kind="ExternalInput"
    )
    q_d = nc.dram_tensor("q", [n_loc, D], mybir.dt.float32, kind="ExternalInput")
    table_d = nc.dram_tensor(
        "table", [m, TE], mybir.dt.float32, kind="ExternalInput"
    )
    out_d = nc.dram_tensor("out", [n_loc, D], mybir.dt.float32, kind="ExternalOutput")

    AF = mybir.ActivationFunctionType
    OP = mybir.AluOpType
    SEG = m // NSEG

    with tile.TileContext(nc) as tc:
        with (
            tc.tile_pool(name="const", bufs=1) as cpool,
            tc.tile_pool(name="score", bufs=2) as spool,
            tc.tile_pool(name="psum", bufs=2, space="PSUM") as ppool,
            tc.tile_pool(name="gath", bufs=3) as gpool,
            tc.tile_pool(name="small", bufs=8) as mpool,
        ):
            comb = cpool.tile([P, mg + n_loc], mybir.dt.bfloat16)
            for g in range(G):
                nc.gpsimd.dma_start(
                    out=comb[32 * g : 32 * g + KR, :],
                    in_=comb_d[KR * g : KR * g + KR, :],
                )
            q_all = cpool.tile([P, nt, D], mybir.dt.float32)
            nc.gpsimd.dma_start(
                out=q_all[:, :, :],
                in_=q_d[:, :].rearrange("(t p) d -> p t d", p=P),
            )
            out_all = cpool.tile([P, nt, D], mybir.dt.float32)
            nq_all = cpool.tile([P, nt, D], mybir.dt.float32)
            nc.gpsimd.tensor_scalar(
                nq_all[:, :, :], q_all[:, :, :], -1.0, None, op0=OP.mult
            )

            for t in range(nt):
                # --- packed scores: PE -> PSUM -> (DMA) -> SBUF ---
                score = spool.tile([P, m], mybir.dt.float32, tag="score")
                for cg in range(m // (BANKS * CH)):
                    ps = ppool.tile([P, BANKS * CH], mybir.dt.float32, tag="ps")
                    for i in range(BANKS):
                        c = cg * BANKS + i
                        g, lc = c // ncg, c % ncg
                        pb = 32 * g
                        nc.tensor.matmul(
                            ps[:, i * CH : (i + 1) * CH],
                            lhsT=comb[pb : pb + KR, mg + t * P : mg + (t + 1) * P],
                            rhs=comb[pb : pb + KR, lc * CH : (lc + 1) * CH],
                            start=True, stop=True,
                            tile_position=(pb, 0),
                        )
                    nc.scalar.copy(
                        score[:, cg * BANKS * CH : (cg + 1) * BANKS * CH],
                        ps[:, :],
                    )

                # --- segmented max8: 32 packed candidates per query ---
                cands = mpool.tile([P, CAND], mybir.dt.float32, tag="cands")
                for s in range(NSEG):
                    nc.vector.max(
                        cands[:, 8 * s : 8 * s + 8],
                        score[:, s * SEG : (s + 1) * SEG],
                    )

                # --- index extraction: j = cands - 2^14*trunc(cands*2^-14) ---
                # (Pool holds the mlp library for dma_gather; InstTensorTensor
                # needs the standard library, so tensor+tensor ops go to DVE.)
                uf = mpool.tile([P, CAND], mybir.dt.float32, tag="uf")
                nc.vector.tensor_scalar_mul(uf[:, :], cands[:, :], 2.0**-14)
                ti = mpool.tile([P, CAND], mybir.dt.int32, tag="ti")
                nc.vector.tensor_copy(ti[:, :], uf[:, :])
                tf = mpool.tile([P, CAND], mybir.dt.float32, tag="tf")
                nc.vector.tensor_copy(tf[:, :], ti[:, :])
                jf = mpool.tile([P, CAND], mybir.dt.float32, tag="jf")
                nc.vector.tensor_scalar(
                    jf[:, :], tf[:, :], -(2.0**14), None, op0=OP.mult
                )
                nc.vector.tensor_add(jf[:, :], jf[:, :], cands[:, :])
                # float->int cast mode (trunc vs nearest) is ALU-dependent;
                # fix up a one-quantum overshoot so jf lands in [0, 2^14).
                fx = mpool.tile([P, CAND], mybir.dt.float32, tag="fx")
                nc.vector.tensor_scalar(
                    fx[:, :], jf[:, :], 0.0, 2.0**14, op0=OP.is_lt, op1=OP.mult
                )
                nc.vector.tensor_add(jf[:, :], jf[:, :], fx[:, :])
                idx32 = mpool.tile([P, CAND], mybir.dt.uint32, tag="idx32")
                nc.vector.tensor_copy(idx32[:, :], jf[:, :])

                if stage < 3:
                    nc.vector.tensor_copy(out_all[:, t, :], jf[:, 0:D])
                    continue

                # --- gather each candidate's (pts, flow) row: one offset per
                # partition per indirect DMA (the multi-offset form crashes
                # the exec unit on HW) ---
                gout = gpool.tile([P, CAND, TE], mybir.dt.float32, tag="gout")
                for k in range(CAND):
                    nc.gpsimd.indirect_dma_start(
                        out=gout[:, k, :],
                        out_offset=None,
                        in_=table_d[:, :],
                        in_offset=bass.IndirectOffsetOnAxis(
                            ap=idx32[:, k : k + 1], axis=0
                        ),
                    )

                # --- exact d^2 refinement ---
                dsq = mpool.tile([P, CAND, D], mybir.dt.float32, tag="dsq")
                for c in range(D):
                    nc.scalar.activation(
                        dsq[:, :, c], gout[:, :, c], AF.Square,
                        bias=nq_all[:, t, c : c + 1],
                    )
                d2 = mpool.tile([P, CAND], mybir.dt.float32, tag="d2")
                nc.vector.tensor_reduce(
                    d2[:, :], dsq[:, :, :], op=OP.add, axis=mybir.AxisListType.X
                )
                # nd2 = -(d2 + 1e-8) ; top-8 by exact distance
                nd2 = mpool.tile([P, CAND], mybir.dt.float32, tag="nd2")
                nc.vector.tensor_scalar(
                    nd2[:, :], d2[:, :], -1.0, -1e-8, op0=OP.mult, op1=OP.add
                )
                v8 = mpool.tile([P, K], mybir.dt.float32, tag="v8")
                nc.vector.max(v8[:, :], nd2[:, :])
                # mask of the 8 nearest; w = mask * 1/nd2 (negative weights;
                # the normalization below cancels the sign)
                mask = mpool.tile([P, CAND], mybir.dt.float32, tag="mask")
                nc.vector.tensor_scalar(
                    mask[:, :], nd2[:, :], v8[:, 7:8], None, op0=OP.is_ge
                )
                rec = mpool.tile([P, CAND], mybir.dt.float32, tag="rec")
                nc.vector.reciprocal(rec[:, :], nd2[:, :])
                acc = mpool.tile([P, 4], mybir.dt.float32, tag="acc")
                w = mpool.tile([P, CAND], mybir.dt.float32, tag="w")
                nc.vector.tensor_mul(w[:, :], mask[:, :], rec[:, :])
                nc.vector.tensor_reduce(
                    acc[:, 3:4], w[:, :], op=OP.add, axis=mybir.AxisListType.X
                )
                for c in range(D):
                    pr = mpool.tile([P, CAND], mybir.dt.float32, tag=f"pr{c}")
                    nc.vector.tensor_mul(pr[:, :], w[:, :], gout[:, :, D + c])
                    nc.vector.tensor_reduce(
                        acc[:, c : c + 1], pr[:, :], op=OP.add,
                        axis=mybir.AxisListType.X,
                    )
                wr = mpool.tile([P, 1], mybir.dt.float32, tag="wr")
                nc.vector.reciprocal(wr[:, :], acc[:, 3:4])
                nc.vector.tensor_scalar(
                    out_all[:, t, :], acc[:, 0:D], wr[:, 0:1], None, op0=OP.mult
                )

            nc.gpsimd.dma_start(
                out=out_d[:, :].rearrange("(t p) d -> p t d", p=P),
                in_=out_all[:, :, :],
            )

    # InstTensorTensorReduce is an InstISA subclass: emit its ISA bytes
    # (Bacc runs this pass; plain Bass skips it).
    mybir.codegen_inst_isa_subclasses(nc)

    if split:
        _split_waits(nc)
    return nc


_SPLIT_SEQ = [0]


def _split_waits(nc, limit=1):
    """Move excess sem-waits onto preceding same-engine NOPs.

    Several TRN2 ISA structs accept only a small number of sync-wait
    commands and walrus refuses to split them ("Too many sync wait
    commands"). A NOP carrying one wait is always legal, and a wait
    executed earlier on the same engine is strictly more conservative,
    so this preserves correctness.
    """
    import concourse.mybir as mybir  # noqa: PLC0415
    from concourse.tile_rust import add_dep_helper  # noqa: PLC0415

    for fn in nc.m.functions:
        for b in fn.blocks:
            il = b.instructions
            idx = 0
            while idx < len(il):
                inst = il[idx]
                si = inst.sync_info
                if si is not None and len(si.on_wait) > limit:
                    waits = list(si.on_wait)
                    excess, keep = waits[:-limit], waits[-limit:]
                    inst.sync_info = mybir.SyncInfo(
                        on_wait=keep, on_update=list(si.on_update)
                    )
                    def _safe_dep(a, b):
                        try:
                            add_dep_helper(a, b, True, "waitnop order")
                            return True
                        except ValueError:
                            return False

                    prev = None
                    for k in range(idx - 1, -1, -1):
                        if il[k].engine == inst.engine:
                            prev = il[k]
                            break
                    chain = prev
                    for j, wt in enumerate(excess):
                        _SPLIT_SEQ[0] += 1
                        nop = mybir.InstNoOp(
                            name=f"waitnop-{_SPLIT_SEQ[0]}", ins=[], outs=[]
                        )
                        nop.engine = inst.engine
                        nop.sync_info = mybir.SyncInfo(on_wait=[wt], on_update=[])
                        nc.register_instruction(nop, overwrite=True)
                        if chain is not None:
                            _safe_dep(nop, chain)
                        chain = nop
                        il.insert(idx + j, nop)
                    _safe_dep(inst, chain)
                    idx += len(excess)
                idx += 1


def _morton_interleave(r):
    """Column order for refs: Morton-sort, then deal alternately into the
    two segment halves.  The 8 nearest neighbors of any query are spatially
    clustered -> adjacent in Morton order -> split ~4/4 across halves, so a
    per-half top-8 never drops a true top-8 member."""
    m = r.shape[0]
    lo, hi = r.min(0), r.max(0)
    gcell = np.clip(((r - lo) / (hi - lo + 1e-9) * 1024).astype(np.uint64), 0, 1023)

    def spread(x):
        x = (x | (x << 16)) & 0x030000FF
        x = (x | (x << 8)) & 0x0300F00F
        x = (x | (x << 4)) & 0x030C30C3
        x = (x | (x << 2)) & 0x09249249
        return x

    code = spread(gcell[:, 0]) | (spread(gcell[:, 1]) << 1) | (
        spread(gcell[:, 2]) << 2
    )
    order = np.argsort(code, kind="stable")
    cols = np.empty(m, dtype=np.int64)
    half = m // 2
    cols[:half] = order[0::2]
    cols[half:] = order[1::2]
    return cols  # column j holds original ref cols[j]


def pack_inputs(query_points, ref_points, ref_flow):
    """Host-side input marshalling: shard queries, pack/replicate refs."""
    q = np.ascontiguousarray(np.asarray(query_points, dtype=np.float32))
    r0 = np.ascontiguousarray(np.asarray(ref_points, dtype=np.float32))
    f0 = np.ascontiguousarray(np.asarray(ref_flow, dtype=np.float32))
    n, m = q.shape[0], r0.shape[0]
    n_loc = n // N_CORES
    G = 4
    mg = m // G

    cols = _morton_interleave(r0)
    r = r0[cols]
    f = f0[cols]

    table = np.zeros((m, TE), dtype=np.float32)
    table[:, 0:D] = r
    table[:, D : 2 * D] = f

    r64 = r.astype(np.float64)
    rsq = np.round((r64**2).sum(1))
    rsq_hi = _bf(rsq).astype(np.float64)
    rsq_lo = np.round(rsq - rsq_hi)
    rh = _bf(r64).astype(np.float64)
    rl = _bf(r64 - rh).astype(np.float64)
    jg = np.arange(m, dtype=np.float64)
    j_hi = _bf(jg).astype(np.float64)
    j_lo = jg - j_hi

    q64 = q.astype(np.float64)
    qsq = np.round((q64**2).sum(1))
    qsq_hi = _bf(qsq).astype(np.float64)
    qsq_lo = np.round(qsq - qsq_hi)
    qh = _bf(q64).astype(np.float64)
    ql = _bf(q64 - qh).astype(np.float64)

    in_maps = []
    for core in range(N_CORES):
        sl = slice(core * n_loc, (core + 1) * n_loc)
        comb = np.zeros((G * KR, mg + n_loc), dtype=np.float64)
        for g in range(G):
            ms = slice(g * mg, (g + 1) * mg)
            # P = C2eff - round(d^2)*2^14 + j.  The d^2 terms (scale 2^14,
            # Delta = 1 d^2-unit per quantum) accumulate FIRST at natural
            # fp32 magnitude (~2^29, error ~1e-3 units); adding C1 then
            # quantizes the running sum to the 2^14 quantum ONCE (+-0.5
            # units total), -C1 cancels exactly (Sterbenz), and the iota
            # rows land in the exact low-14-bit window.
            rows_ref = np.zeros((KR, mg))
            rows_q = np.zeros((KR, n_loc))
            rows_ref[0] = 2.0**14; rows_q[0] = -qsq_hi[sl]
            rows_ref[1] = 2.0**14; rows_q[1] = -qsq_lo[sl]
            rows_ref[2] = -rsq_hi[ms] * 2.0**14; rows_q[2] = 1.0
            rows_ref[3] = -rsq_lo[ms] * 2.0**14; rows_q[3] = 1.0
            for c in range(D):
                rows_ref[4 + 3 * c] = rh[ms, c] * 2.0**8
                rows_q[4 + 3 * c] = qh[sl, c] * 2.0**7
                rows_ref[5 + 3 * c] = rl[ms, c] * 2.0**8
                rows_q[5 + 3 * c] = qh[sl, c] * 2.0**7
                rows_ref[6 + 3 * c] = rh[ms, c] * 2.0**8
                rows_q[6 + 3 * c] = ql[sl, c] * 2.0**7
            rows_ref[13] = C1; rows_q[13] = 1.0
            rows_ref[14] = -C1; rows_q[14] = 1.0
            rows_ref[15] = 2.0**24; rows_q[15] = 1.0
            rows_ref[16] = -24.0 * 2.0**14; rows_q[16] = 1.0
            rows_ref[17] = j_hi[ms]; rows_q[17] = 1.0
            rows_ref[18] = j_lo[ms]; rows_q[18] = 1.0
            comb[KR * g : KR * (g + 1), :mg] = rows_ref
            comb[KR * g : KR * (g + 1), mg:] = rows_q
        in_maps.append(
            {"comb": _bf(comb), "q": q[sl], "table": table}
        )
    return in_maps


_NC_CACHE = {}


def _get_module(n_loc, m):
    """Build + verify-compile the module.

    The Tile scheduler is process-nondeterministic (rust hash seeds) and
    some schedules emit more sem-waits on an instruction than its ISA
    struct allows, which walrus rejects. walrus is deterministic given a
    BIR, so: rebuild until a test-compile passes, then reuse that module
    for the real run (same BIR -> same walrus outcome).
    """
    import tempfile

    from concourse.bass_utils import compile_bir_kernel

    key = (n_loc, m)
    if key not in _NC_CACHE:
        last = None
        for _attempt in range(12):
            nc = build_module(n_loc, m)
            try:
                with tempfile.TemporaryDirectory() as td:
                    compile_bir_kernel(nc.to_json_bytes(), td)
                _NC_CACHE[key] = nc
                break
            except Exception as e:  # noqa: BLE001 — retry on compile flake
                last = e
        else:
            raise RuntimeError(f"no schedule compiled after 12 tries: {last}")
    return _NC_CACHE[key]


def run_hw(query_points, ref_points, ref_flow, trace=False):
    in_maps = pack_inputs(query_points, ref_points, ref_flow)
    n = np.asarray(query_points).shape[0]
    m = np.asarray(ref_points).shape[0]
    nc = _get_module(n // N_CORES, m)
    res = run_bass_kernel_spmd(
        nc, in_maps, core_ids=list(range(N_CORES)), trace=trace
    )
    out = np.concatenate([r["out"] for r in res.results], axis=0)
    return out, res


def kernel(query_points, ref_points, ref_flow, power, k):
    assert int(power) == 2 and int(k) == K
    out, _ = run_hw(query_points, ref_points, ref_flow, trace=False)
    return out

